# revision 5
# baseline (speedup 1.0000x reference)
"""DBRX MoE experts kernel for 8 Trainium2 NeuronCores.

Strategy (expert-parallel with host-side token dispatch):
  - Host computes the (cheap) router: softmax over 16 experts, top-4,
    renormalized gates.  Tokens are gathered per expert.
  - Each core gets NG=2 expert "groups" (16 experts / 8 cores).  Experts are
    sorted by token count: the 8 largest go in group 0, the 8 smallest in
    group 1, and each group's tokens are packed into MG tiles of T_g tokens
    (zero padded, T sized per group to the largest expert in it).  The
    expert's weights are loaded once per group and reused across its tiles.
  - Device (SPMD, one program on all 8 cores) runs the expert FFN:
    h = wsT.T @ x (both halves), act = silu(h1)*h2, y = w2T.T @ act.
    All matmuls in bfloat16 (1 cycle/row, FWL weight loads at 2 elem/cycle,
    half the DMA bytes of fp32; end-to-end ~4e-3 rel err vs 2e-2 budget).
  - Host applies gates and scatter-adds item outputs into the [T, D] output.
    Only the FFN (97% of the FLOPs) runs on device; the dense 16-expert
    reference computation is avoided entirely (4x FLOP saving via top-4).

Self-contained: hardcodes T=4096 tokens, D=1024, I=2048, E=16, top_k=4,
8 cores.
"""

import sys

if "/opt/trn_rl_repo" not in sys.path:
    sys.path.insert(0, "/opt/trn_rl_repo")

import ml_dtypes
import numpy as np

import concourse.bacc as bacc
import concourse.mybir as mybir
import concourse.tile as tile
from concourse.bass_utils import run_bass_kernel_spmd

TOP_K = 4
N_CORES = 8
D = 1024
I = 2048
E = 16
DC = D // 128  # 8 contraction chunks for mm1 / output blocks for mm2
IC = I // 128  # 16 intermediate blocks
CB = 2 * I // 128  # 32 column blocks of ws

BF16 = ml_dtypes.bfloat16

TRACE = False
LAST_EXEC_NS = None

_compiled = {}  # shapes tuple -> nc


def _build_program(shapes):
    """shapes: tuple of (MG, T) per group (one group = one expert)."""
    bf16 = mybir.dt.bfloat16
    f32 = mybir.dt.float32
    NG = len(shapes)
    nc = bacc.Bacc("TRN2", target_bir_lowering=False, debug=False, num_devices=N_CORES)

    xTs, yTs = [], []
    for g, (MG, T) in enumerate(shapes):
        xTs.append(
            nc.dram_tensor(f"xT{g}", [MG, 128, DC, T], bf16, kind="ExternalInput")
        )
        yTs.append(
            nc.dram_tensor(f"yT{g}", [MG, DC, 128, T], bf16, kind="ExternalOutput")
        )
    wsT = nc.dram_tensor("wsT", [NG, CB, 128, DC, 128], bf16, kind="ExternalInput")
    w2T = nc.dram_tensor("w2T", [NG, DC, 128, IC, 128], bf16, kind="ExternalInput")

    with tile.TileContext(nc) as tc:
        with (
            tc.tile_pool(name="xp", bufs=max(mg for mg, _ in shapes)) as xp,
            tc.tile_pool(name="wp", bufs=4) as wp,
            tc.tile_pool(name="w2p", bufs=3) as w2p,
            tc.tile_pool(name="actp", bufs=max(mg for mg, _ in shapes)) as actp,
            tc.tile_pool(name="sp", bufs=3) as sp,
            tc.tile_pool(name="pp", bufs=6, space="PSUM") as pp,
            tc.tile_pool(name="pp2", bufs=2, space="PSUM") as pp2,
        ):
            Tmax = max(t for _, t in shapes)
            for g, (MG, T) in enumerate(shapes):
                # Issue order tuned for startup: the first matmul needs w1t0
                # and the j=0 x chunks, so those go first, interleaved across
                # the two HWDGE issue engines (sync/scalar); v1t0 (needed only
                # after the 8 ps1 matmuls) and the j>0 tiles follow.
                w1t0 = wp.tile([128, DC, 128], bf16, tag="ws", name=f"w1t0_{g}")
                nc.sync.dma_start(w1t0[:], wsT.ap()[g, 0])
                xts = []
                acts = []
                for j in range(MG):
                    # allocate at Tmax with a shared tag (slot reuse across
                    # groups), slice to this group's T
                    xt = xp.tile(
                        [128, DC, Tmax], bf16, tag="x", name=f"x_{g}_{j}"
                    )[:, :, :T]
                    xts.append(xt)
                    act = actp.tile(
                        [128, IC, Tmax], bf16, tag="act", name=f"act_{g}_{j}"
                    )[:, :, :T]
                    acts.append(act)
                # j=0 chunks split fine (one per dc) and alternated across
                # issue engines so the first chunks land quickly
                for dc in range(DC):
                    eng = nc.scalar if dc % 2 else nc.sync
                    eng.dma_start(
                        xts[0][:, dc : dc + 1], xTs[g].ap()[0, :, dc : dc + 1]
                    )
                v1t0 = wp.tile([128, DC, 128], bf16, tag="ws", name=f"v1t0_{g}")
                nc.scalar.dma_start(v1t0[:], wsT.ap()[g, IC])
                for j in range(1, MG):
                    # split into 4 chunks: one big DMA bottlenecks on a single
                    # DMA queue; chunk DMAs spread across queues.
                    for k, dc in enumerate(range(0, DC, 2)):
                        eng = nc.scalar if (j + k) % 2 else nc.sync
                        eng.dma_start(
                            xts[j][:, dc : dc + 2], xTs[g].ap()[j, :, dc : dc + 2]
                        )

                # mm1 + SwiGLU: weights outer, token tiles inner (weight reuse)
                for ip in range(IC):
                    if ip == 0:
                        w1t, v1t = w1t0, v1t0
                    else:
                        w1t = wp.tile([128, DC, 128], bf16, tag="ws")
                        nc.sync.dma_start(w1t[:], wsT.ap()[g, ip])
                        v1t = wp.tile([128, DC, 128], bf16, tag="ws")
                        nc.sync.dma_start(v1t[:], wsT.ap()[g, IC + ip])
                    for j in range(MG):
                        ps1 = pp.tile([128, T], f32, tag="h")
                        ps2 = pp.tile([128, T], f32, tag="h")
                        for dc in range(DC):
                            nc.tensor.matmul(
                                ps1[:], w1t[:, dc], xts[j][:, dc],
                                start=(dc == 0), stop=(dc == DC - 1),
                            )
                        for dc in range(DC):
                            nc.tensor.matmul(
                                ps2[:], v1t[:, dc], xts[j][:, dc],
                                start=(dc == 0), stop=(dc == DC - 1),
                            )
                        st = sp.tile([128, T], f32, tag="silu")
                        nc.scalar.activation(
                            st[:], ps1[:], mybir.ActivationFunctionType.Silu
                        )
                        nc.vector.tensor_mul(acts[j][:, ip], st[:], ps2[:])

                # mm2: w2 slabs outer, token tiles inner (weight reuse)
                last_g = g == len(shapes) - 1
                for db in range(DC):
                    w2t = w2p.tile([128, IC, 128], bf16, tag="w2")
                    nc.sync.dma_start(w2t[:], w2T.ap()[g, db])
                    for j in range(MG):
                        # last slab of the kernel: compute/copy/DMA in column
                        # halves so the writeback overlaps the final matmuls
                        halves = (
                            2 if last_g and db == DC - 1 and j == MG - 1 else 1
                        )
                        ps3 = pp2.tile([128, T], f32, tag="y")
                        Th = T // halves
                        for h in range(halves):
                            sl = slice(h * Th, (h + 1) * Th)
                            for ic in range(IC):
                                nc.tensor.matmul(
                                    ps3[:, sl], w2t[:, ic], acts[j][:, ic, sl],
                                    start=(ic == 0), stop=(ic == IC - 1),
                                )
                            ot = sp.tile([128, Th], bf16, tag="yout")
                            nc.any.tensor_copy(ot[:], ps3[:, sl])
                            eng = nc.scalar if h % 2 else nc.sync
                            eng.dma_start(yTs[g].ap()[j, db, :, sl], ot[:])
    nc.compile()
    return nc


def _routing(x, rw):
    logits = x @ rw.T
    m = logits.max(-1, keepdims=True)
    p = np.exp(logits - m)
    p /= p.sum(-1, keepdims=True)
    topk_idx = np.argpartition(-p, TOP_K - 1, axis=-1)[:, :TOP_K]
    topk_val = np.take_along_axis(p, topk_idx, -1)
    topk_val = topk_val / topk_val.sum(-1, keepdims=True)
    return topk_idx, topk_val


def _group_shape(cmax):
    """Pick (MG, T) so MG*T >= cmax, T in [128, 512] mult of 4 (even halves),
    minimizing MG*T (tie: prefer fewer/larger tiles)."""
    best = None
    for MG in range(1, 17):
        T = -(-cmax // MG) if cmax else 128
        T = (T + 3) // 4 * 4
        if T > 512:
            continue
        T = max(T, 128)
        if best is None or MG * T < best[0]:
            best = (MG * T, MG, T)
    assert best is not None
    return best[1], best[2]


def _tile_ws(ws_e):
    # [cb, p, dc, col] = ws_e[cb*128+col, dc*128+p]
    return np.ascontiguousarray(
        ws_e.reshape(CB, 128, DC, 128).transpose(0, 3, 2, 1)
    )


def _tile_w2(w2_e):
    # [db, p, ic, col] = w2_e[db*128+col, ic*128+p]
    return np.ascontiguousarray(
        w2_e.reshape(DC, 128, IC, 128).transpose(0, 3, 2, 1)
    )


def kernel(hidden_states, router_w, ws, w2s):
    global LAST_EXEC_NS
    x = np.ascontiguousarray(np.asarray(hidden_states, dtype=np.float32))
    rw = np.asarray(router_w, dtype=np.float32)
    ws = np.asarray(ws, dtype=np.float32)
    w2s = np.asarray(w2s, dtype=np.float32)
    T_tok = x.shape[0]

    topk_idx, topk_val = _routing(x, rw)

    expert_tok = []
    expert_gate = []
    for e in range(E):
        hit = topk_idx == e
        rows = np.nonzero(hit.any(-1))[0]
        gv = np.where(hit[rows], topk_val[rows], 0.0).sum(-1).astype(np.float32)
        expert_tok.append(rows)
        expert_gate.append(gv)

    counts = np.array([len(t) for t in expert_tok])
    NG = -(-E // N_CORES)  # 2
    # sort experts by count desc; group g holds ranks [g*8, g*8+8)
    order = np.argsort(-counts, kind="stable")
    groups = [order[g * N_CORES : (g + 1) * N_CORES] for g in range(NG)]
    shapes = tuple(
        _group_shape(int(counts[grp].max()) if len(grp) else 0) for grp in groups
    )

    if shapes not in _compiled:
        _compiled[shapes] = _build_program(shapes)
    nc = _compiled[shapes]

    x16 = x.astype(BF16)
    in_maps = []
    for c in range(N_CORES):
        m = {}
        wsT_b = np.empty((NG, CB, 128, DC, 128), dtype=BF16)
        w2T_b = np.empty((NG, DC, 128, IC, 128), dtype=BF16)
        for g, (MG, T) in enumerate(shapes):
            e = int(groups[g][c])
            wsT_b[g] = _tile_ws(ws[e].astype(BF16))
            w2T_b[g] = _tile_w2(w2s[e].astype(BF16))
            xT_b = np.zeros((MG, 128, DC, T), dtype=BF16)
            toks = expert_tok[e]
            for j in range(MG):
                seg = toks[j * T : (j + 1) * T]
                n = len(seg)
                if n == 0:
                    continue
                xT_b[j, :, :, :n] = x16[seg].reshape(n, DC, 128).transpose(2, 1, 0)
            m[f"xT{g}"] = xT_b
        m["wsT"] = wsT_b
        m["w2T"] = w2T_b
        in_maps.append(m)

    res = run_bass_kernel_spmd(
        nc, in_maps, core_ids=list(range(N_CORES)), trace=TRACE
    )
    LAST_EXEC_NS = res.exec_time_ns

    out = np.zeros((T_tok, D), dtype=np.float32)
    for g, (MG, T) in enumerate(shapes):
        for c in range(N_CORES):
            e = int(groups[g][c])
            toks = expert_tok[e]
            gates = expert_gate[e]
            yT_c = np.asarray(res.results[c][f"yT{g}"]).astype(np.float32)
            for j in range(MG):
                seg = toks[j * T : (j + 1) * T]
                n = len(seg)
                if n == 0:
                    break
                y_item = yT_c[j].transpose(2, 0, 1).reshape(T, D)[:n]
                out[seg] += gates[j * T : (j + 1) * T][:, None] * y_item
    return out


# revision 8
# speedup vs baseline: 1.0021x; 1.0021x over previous
"""DBRX MoE experts kernel for 8 Trainium2 NeuronCores.

Strategy (expert-parallel with host-side token dispatch):
  - Host computes the (cheap) router: softmax over 16 experts, top-4,
    renormalized gates.  Tokens are gathered per expert.
  - Each core gets NG=2 expert "groups" (16 experts / 8 cores).  Experts are
    sorted by token count: the 8 largest go in group 0, the 8 smallest in
    group 1, and each group's tokens are packed into MG tiles of T_g tokens
    (zero padded, T sized per group to the largest expert in it).  The
    expert's weights are loaded once per group and reused across its tiles.
  - Device (SPMD, one program on all 8 cores) runs the expert FFN:
    h = wsT.T @ x (both halves), act = silu(h1)*h2, y = w2T.T @ act.
    All matmuls in bfloat16 (1 cycle/row, FWL weight loads at 2 elem/cycle,
    half the DMA bytes of fp32; end-to-end ~4e-3 rel err vs 2e-2 budget).
  - Host applies gates and scatter-adds item outputs into the [T, D] output.
    Only the FFN (97% of the FLOPs) runs on device; the dense 16-expert
    reference computation is avoided entirely (4x FLOP saving via top-4).

Self-contained: hardcodes T=4096 tokens, D=1024, I=2048, E=16, top_k=4,
8 cores.
"""

import sys

if "/opt/trn_rl_repo" not in sys.path:
    sys.path.insert(0, "/opt/trn_rl_repo")

import ml_dtypes
import numpy as np

import concourse.bacc as bacc
import concourse.mybir as mybir
import concourse.tile as tile
from concourse.bass_utils import run_bass_kernel_spmd

TOP_K = 4
N_CORES = 8
D = 1024
I = 2048
E = 16
DC = D // 128  # 8 contraction chunks for mm1 / output blocks for mm2
IC = I // 128  # 16 intermediate blocks
CB = 2 * I // 128  # 32 column blocks of ws

BF16 = ml_dtypes.bfloat16

TRACE = False
LAST_EXEC_NS = None

_compiled = {}  # shapes tuple -> nc


def _build_program(shapes):
    """shapes: tuple of (MG, T) per group (one group = one expert)."""
    bf16 = mybir.dt.bfloat16
    f32 = mybir.dt.float32
    NG = len(shapes)
    nc = bacc.Bacc("TRN2", target_bir_lowering=False, debug=False, num_devices=N_CORES)

    xTs, yTs = [], []
    for g, (MG, T) in enumerate(shapes):
        xTs.append(
            nc.dram_tensor(f"xT{g}", [MG, 128, DC, T], bf16, kind="ExternalInput")
        )
        yTs.append(
            nc.dram_tensor(f"yT{g}", [MG, DC, 128, T], bf16, kind="ExternalOutput")
        )
    wsT = nc.dram_tensor("wsT", [NG, CB, 128, DC, 128], bf16, kind="ExternalInput")
    w2T = nc.dram_tensor("w2T", [NG, DC, 128, IC, 128], bf16, kind="ExternalInput")

    with tile.TileContext(nc) as tc:
        with (
            tc.tile_pool(name="xp", bufs=max(mg for mg, _ in shapes)) as xp,
            tc.tile_pool(name="wp", bufs=4) as wp,
            tc.tile_pool(name="w2p", bufs=3) as w2p,
            tc.tile_pool(name="actp", bufs=max(mg for mg, _ in shapes)) as actp,
            tc.tile_pool(name="sp", bufs=3) as sp,
            tc.tile_pool(name="pp", bufs=6, space="PSUM") as pp,
            tc.tile_pool(name="pp2", bufs=2, space="PSUM") as pp2,
        ):
            Tmax = max(t for _, t in shapes)
            for g, (MG, T) in enumerate(shapes):
                # Issue order tuned for startup: the first matmul needs w1t0
                # and the j=0 x chunks, so those go first, interleaved across
                # the two HWDGE issue engines (sync/scalar); v1t0 (needed only
                # after the 8 ps1 matmuls) and the j>0 tiles follow.
                w1t0 = wp.tile([128, DC, 128], bf16, tag="ws", name=f"w1t0_{g}")
                nc.sync.dma_start(w1t0[:], wsT.ap()[g, 0])
                xts = []
                acts = []
                for j in range(MG):
                    # allocate at Tmax with a shared tag (slot reuse across
                    # groups), slice to this group's T
                    xt = xp.tile(
                        [128, DC, Tmax], bf16, tag="x", name=f"x_{g}_{j}"
                    )[:, :, :T]
                    xts.append(xt)
                    act = actp.tile(
                        [128, IC, Tmax], bf16, tag="act", name=f"act_{g}_{j}"
                    )[:, :, :T]
                    acts.append(act)
                # j=0 chunks alternated across issue engines so the first
                # chunks land quickly; v1t0 (first needed after the 8 ps1
                # matmuls) issues in parallel with the k2/k3 chunks
                for k, dc in enumerate(range(0, DC, 2)):
                    eng = nc.scalar if k % 2 else nc.sync
                    eng.dma_start(
                        xts[0][:, dc : dc + 2], xTs[g].ap()[0, :, dc : dc + 2]
                    )
                v1t0 = wp.tile([128, DC, 128], bf16, tag="ws", name=f"v1t0_{g}")
                nc.scalar.dma_start(v1t0[:], wsT.ap()[g, IC])
                for j in range(1, MG):
                    # split into 4 chunks: one big DMA bottlenecks on a single
                    # DMA queue; chunk DMAs spread across queues.
                    for k, dc in enumerate(range(0, DC, 2)):
                        eng = nc.scalar if (j + k) % 2 else nc.sync
                        eng.dma_start(
                            xts[j][:, dc : dc + 2], xTs[g].ap()[j, :, dc : dc + 2]
                        )

                # mm1 + SwiGLU: weights outer, token tiles inner (weight reuse)
                for ip in range(IC):
                    if ip == 0:
                        w1t, v1t = w1t0, v1t0
                    else:
                        w1t = wp.tile([128, DC, 128], bf16, tag="ws")
                        nc.sync.dma_start(w1t[:], wsT.ap()[g, ip])
                        v1t = wp.tile([128, DC, 128], bf16, tag="ws")
                        nc.sync.dma_start(v1t[:], wsT.ap()[g, IC + ip])
                    for j in range(MG):
                        ps1 = pp.tile([128, T], f32, tag="h")
                        ps2 = pp.tile([128, T], f32, tag="h")
                        for dc in range(DC):
                            nc.tensor.matmul(
                                ps1[:], w1t[:, dc], xts[j][:, dc],
                                start=(dc == 0), stop=(dc == DC - 1),
                            )
                        for dc in range(DC):
                            nc.tensor.matmul(
                                ps2[:], v1t[:, dc], xts[j][:, dc],
                                start=(dc == 0), stop=(dc == DC - 1),
                            )
                        st = sp.tile([128, T], f32, tag="silu")
                        nc.scalar.activation(
                            st[:], ps1[:], mybir.ActivationFunctionType.Silu
                        )
                        nc.vector.tensor_mul(acts[j][:, ip], st[:], ps2[:])

                # mm2: w2 slabs outer, token tiles inner (weight reuse)
                last_g = g == len(shapes) - 1
                for db in range(DC):
                    w2t = w2p.tile([128, IC, 128], bf16, tag="w2")
                    nc.sync.dma_start(w2t[:], w2T.ap()[g, db])
                    for j in range(MG):
                        # last slab of the kernel: compute/copy/DMA in column
                        # halves (separate PSUM tiles — same tile would add a
                        # false matmul->copy dependency) so the writeback
                        # overlaps the final matmuls
                        halves = (
                            2 if last_g and db == DC - 1 and j == MG - 1 else 1
                        )
                        Th = T // halves
                        for h in range(halves):
                            sl = slice(h * Th, (h + 1) * Th)
                            ps3 = pp2.tile([128, Th], f32, tag="y")
                            for ic in range(IC):
                                nc.tensor.matmul(
                                    ps3[:], w2t[:, ic], acts[j][:, ic, sl],
                                    start=(ic == 0), stop=(ic == IC - 1),
                                )
                            ot = sp.tile([128, Th], bf16, tag="yout")
                            nc.any.tensor_copy(ot[:], ps3[:])
                            eng = nc.scalar if h % 2 else nc.sync
                            eng.dma_start(yTs[g].ap()[j, db, :, sl], ot[:])
    nc.compile()
    return nc


def _routing(x, rw):
    logits = x @ rw.T
    m = logits.max(-1, keepdims=True)
    p = np.exp(logits - m)
    p /= p.sum(-1, keepdims=True)
    topk_idx = np.argpartition(-p, TOP_K - 1, axis=-1)[:, :TOP_K]
    topk_val = np.take_along_axis(p, topk_idx, -1)
    topk_val = topk_val / topk_val.sum(-1, keepdims=True)
    return topk_idx, topk_val


def _group_shape(cmax):
    """Pick (MG, T) so MG*T >= cmax, T in [128, 512] mult of 4 (even halves),
    minimizing MG*T (tie: prefer fewer/larger tiles)."""
    best = None
    for MG in range(1, 17):
        T = -(-cmax // MG) if cmax else 128
        T = (T + 3) // 4 * 4
        if T > 512:
            continue
        T = max(T, 128)
        if best is None or MG * T < best[0]:
            best = (MG * T, MG, T)
    assert best is not None
    return best[1], best[2]


def _tile_ws(ws_e):
    # [cb, p, dc, col] = ws_e[cb*128+col, dc*128+p]
    return np.ascontiguousarray(
        ws_e.reshape(CB, 128, DC, 128).transpose(0, 3, 2, 1)
    )


def _tile_w2(w2_e):
    # [db, p, ic, col] = w2_e[db*128+col, ic*128+p]
    return np.ascontiguousarray(
        w2_e.reshape(DC, 128, IC, 128).transpose(0, 3, 2, 1)
    )


NGROUPS = 4  # expert pieces per core: splits experts for load balance


def _plan(counts, n_groups):
    """Split the E experts' token counts into n_groups*N_CORES near-equal
    pieces (each piece = contiguous token range of one expert), sort pieces
    by size, and pack rank-consecutive pieces into groups of N_CORES slots.
    Returns (shapes, groups): shapes[g] = (MG, T); groups[g][c] =
    (expert, start, size) for core c."""
    nslots = n_groups * N_CORES
    n = np.ones(E, dtype=np.int64)
    while n.sum() < nslots:
        piece = -(-counts // n)
        n[int(np.argmax(piece))] += 1
    pieces = []
    for e in range(E):
        k = int(n[e])
        base, rem = divmod(int(counts[e]), k)
        off = 0
        for i in range(k):
            sz = base + (1 if i < rem else 0)
            pieces.append((sz, e, off))
            off += sz
    pieces.sort(key=lambda p: (-p[0], p[1], p[2]))
    groups = [
        [(e, off, sz) for sz, e, off in pieces[g * N_CORES : (g + 1) * N_CORES]]
        for g in range(n_groups)
    ]
    shapes = tuple(
        _group_shape(max(sz for _, _, sz in grp)) for grp in groups
    )
    return shapes, groups


def kernel(hidden_states, router_w, ws, w2s):
    global LAST_EXEC_NS
    x = np.ascontiguousarray(np.asarray(hidden_states, dtype=np.float32))
    rw = np.asarray(router_w, dtype=np.float32)
    ws = np.asarray(ws, dtype=np.float32)
    w2s = np.asarray(w2s, dtype=np.float32)
    T_tok = x.shape[0]

    topk_idx, topk_val = _routing(x, rw)

    expert_tok = []
    expert_gate = []
    for e in range(E):
        hit = topk_idx == e
        rows = np.nonzero(hit.any(-1))[0]
        gv = np.where(hit[rows], topk_val[rows], 0.0).sum(-1).astype(np.float32)
        expert_tok.append(rows)
        expert_gate.append(gv)

    counts = np.array([len(t) for t in expert_tok])
    shapes, groups = _plan(counts, NGROUPS)
    NG = len(shapes)

    if shapes not in _compiled:
        _compiled[shapes] = _build_program(shapes)
    nc = _compiled[shapes]

    x16 = x.astype(BF16)
    ws16 = {}
    w2s16 = {}
    in_maps = []
    for c in range(N_CORES):
        m = {}
        wsT_b = np.empty((NG, CB, 128, DC, 128), dtype=BF16)
        w2T_b = np.empty((NG, DC, 128, IC, 128), dtype=BF16)
        for g, (MG, T) in enumerate(shapes):
            e, off, sz = groups[g][c]
            if e not in ws16:
                ws16[e] = _tile_ws(ws[e].astype(BF16))
                w2s16[e] = _tile_w2(w2s[e].astype(BF16))
            wsT_b[g] = ws16[e]
            w2T_b[g] = w2s16[e]
            xT_b = np.zeros((MG, 128, DC, T), dtype=BF16)
            toks = expert_tok[e][off : off + sz]
            for j in range(MG):
                seg = toks[j * T : (j + 1) * T]
                nn = len(seg)
                if nn == 0:
                    continue
                xT_b[j, :, :, :nn] = x16[seg].reshape(nn, DC, 128).transpose(2, 1, 0)
            m[f"xT{g}"] = xT_b
        m["wsT"] = wsT_b
        m["w2T"] = w2T_b
        in_maps.append(m)

    res = run_bass_kernel_spmd(
        nc, in_maps, core_ids=list(range(N_CORES)), trace=TRACE
    )
    LAST_EXEC_NS = res.exec_time_ns

    out = np.zeros((T_tok, D), dtype=np.float32)
    for g, (MG, T) in enumerate(shapes):
        for c in range(N_CORES):
            e, off, sz = groups[g][c]
            toks = expert_tok[e][off : off + sz]
            gates = expert_gate[e][off : off + sz]
            yT_c = np.asarray(res.results[c][f"yT{g}"]).astype(np.float32)
            for j in range(MG):
                seg = toks[j * T : (j + 1) * T]
                nn = len(seg)
                if nn == 0:
                    break
                y_item = yT_c[j].transpose(2, 0, 1).reshape(T, D)[:nn]
                out[seg] += gates[j * T : (j + 1) * T][:, None] * y_item
    return out
